# revision 10
# baseline (speedup 1.0000x reference)
"""C2fCIB (dense CNN) Trainium2 kernel — batch-parallel over 8 NeuronCores.

Strategy:
  - 2 images per core; all 1x1 convs are PE matmuls (float32r for the first
    conv reading fp32 x, bf16 elsewhere); BN+SiLU folded into one ScalarE
    activation per PSUM group.
  - Involution: 2 images packed on 128 partitions (2img x 64ch). Dynamic
    per-(img,group) 3x3 kernels built on device (mask*kh tensor_scalar +
    constant selector matmul -> per-partition scalars). The 9 depthwise taps
    run on DVE as tensor_scalar_mul (4x bf16 mode) + tensor_tensor adds over
    a zero-padded 82x84 image copy (plus a 1-shifted copy for alignment).
  - Both CIB residuals are folded algebraically into the final conv's merged
    weights ([W1, W2+W3+W4, W3+W4, W4]) and a 2-K-tile block2 cv1.
"""

import sys

if "/opt/trn_rl_repo" not in sys.path:
    sys.path.insert(0, "/opt/trn_rl_repo")

import numpy as np
import ml_dtypes

import concourse.bass as bass
import concourse.bacc as bacc
import concourse.mybir as mybir
from concourse import tile
from concourse.bass_utils import run_bass_kernel_spmd

F32 = mybir.dt.float32
F32R = mybir.dt.float32r
BF16 = mybir.dt.bfloat16
NP_BF16 = ml_dtypes.bfloat16
AF = mybir.ActivationFunctionType

B, C1, H, W = 16, 256, 80, 80
PX = H * W                      # 6400
NCORES = 8
IMGS = B // NCORES              # 2 images per core
CH = 64                         # CIB hidden channels
G, GC, K = 4, 16, 3
RED = 16
EPS = 1e-5
WP = 82                         # padded row width (even, for bf16 alignment)
HP = 82                         # padded rows

# pixel groups: contiguous convs (PSUM = 4 banks = 2048 fp32)
GRPS = [(0, 2048), (2048, 2048), (4096, 2048), (6144, 256)]
# row-aligned groups for conv1 (writes the padded buffer): (row0, nrows, chunk_rows)
RGRPS = [(0, 24, 6), (24, 24, 6), (48, 24, 6), (72, 8, 4)]
# 3x3 tap offsets, index = dy*3+dx matching kh reshape(G, 3, 3)
TAPS = [(dy, dx) for dy in (-1, 0, 1) for dx in (-1, 0, 1)]


def _fold(p):
    w = np.asarray(p["w"], np.float32)[:, :, 0, 0]
    sc = np.asarray(p["gamma"], np.float32) / np.sqrt(np.asarray(p["var"], np.float32) + EPS)
    bi = np.asarray(p["beta"], np.float32) - np.asarray(p["mean"], np.float32) * sc
    return w, sc, bi


def _dup(v):  # (n,) -> (2n,1) image-duplicated column
    return np.concatenate([v, v]).astype(np.float32)[:, None]


def _blockdiag2(wt):  # wt (k,m) -> (2k,2m) two-image block diagonal
    k, m = wt.shape
    z = np.zeros((2 * k, 2 * m), wt.dtype)
    z[:k, :m] = wt
    z[k:, m:] = wt
    return z


def prep_weights(params):
    """Host-side: fold BN, transpose to lhsT layouts, pack/merge. Returns
    dict name -> np array matching the DRAM tensor decls in build_nc."""
    d = {}
    # main cv1: 256->256
    w, sc, bi = _fold(params["cv1"])
    # lhsT[p, kt, mt, m] = w[mt*128+m, kt*128+p]
    d["w_cv1"] = np.ascontiguousarray(w.reshape(2, 128, 2, 128).transpose(3, 2, 0, 1)).astype(np.float32)
    d["s_cv1"] = np.ascontiguousarray(sc.reshape(2, 128).T).astype(np.float32)
    d["b_cv1"] = np.ascontiguousarray(bi.reshape(2, 128).T).astype(np.float32)

    for i, blk in enumerate(params["m"]):
        w, sc, bi = _fold(blk["cv1"])          # (64,128)
        d[f"w_bcv1_{i}"] = w.T.astype(NP_BF16)                     # (128,64)
        d[f"s_bcv1_{i}"] = _dup(sc)
        d[f"b_bcv1_{i}"] = _dup(bi)
        w, sc, bi = _fold(blk["inv"]["conv1"])  # (64,64)
        d[f"w_c1_{i}"] = _blockdiag2(w.T).astype(NP_BF16)          # (128,128)
        d[f"s_c1_{i}"] = _dup(sc)
        d[f"b_c1_{i}"] = _dup(bi)
        w, sc, bi = _fold(blk["inv"]["kg1"])    # (16,64)
        d[f"w_kg1_{i}"] = _blockdiag2((w / PX).T).astype(np.float32)  # (128,32), mean fold
        d[f"s_kg1_{i}"] = _dup(sc)              # (32,1)
        d[f"b_kg1_{i}"] = _dup(bi)
        w, sc, bi = _fold(blk["inv"]["kg2"])    # (36,16)
        d[f"w_kg2_{i}"] = _blockdiag2(w.T).astype(np.float32)      # (32,72)
        d[f"s_kg2_{i}"] = _dup(sc)              # (72,1)
        d[f"b_kg2_{i}"] = _dup(bi)
        w, sc, bi = _fold(blk["inv"]["conv2"])  # (64,64)
        d[f"w_c2_{i}"] = _blockdiag2(w.T).astype(NP_BF16)          # (128,128)
        d[f"s_c2_{i}"] = _dup(sc)
        d[f"b_c2_{i}"] = _dup(bi)
        w, sc, bi = _fold(blk["cv2"])           # (128,64)
        # duplicated on both partition halves so lhsT base matches rhs (per-image slice of packed u)
        d[f"w_bcv2_{i}"] = np.concatenate([w.T, w.T], axis=0).astype(NP_BF16)  # (128,128)
        d[f"s_bcv2_{i}"] = sc.astype(np.float32)[:, None]          # (128,1)
        d[f"b_bcv2_{i}"] = bi.astype(np.float32)[:, None]

    # kernel expansion constants
    j = np.arange(72)
    t = np.arange(9)
    d["mask72"] = (j[:, None] % 9 == t[None, :]).astype(np.float32)      # (72,9)
    c = np.arange(128)
    sel = ((j[:, None] // 36 == c[None, :] // 64)
           & ((j[:, None] % 36) // 9 == (c[None, :] % 64) // 16))
    d["sel"] = sel.astype(np.float32)                                     # (72,128)

    # final cv2: 512->256 with residual-folded merged K tiles
    w, sc, bi = _fold(params["cv2"])            # (256, 512)
    w1, w2, w3, w4 = w[:, 0:128], w[:, 128:256], w[:, 256:384], w[:, 384:512]
    tiles = [w1, w2 + w3 + w4, w3 + w4, w4]
    # lhsT[p, kt, mt, m] = tiles[kt][mt*128+m, p]
    arr = np.stack([tt.reshape(2, 128, 128).transpose(2, 0, 1) for tt in tiles])  # (4,128,2,128)
    d["w_cv2f"] = np.ascontiguousarray(arr.transpose(1, 0, 2, 3)).astype(NP_BF16)
    d["s_cv2f"] = np.ascontiguousarray(sc.reshape(2, 128).T).astype(np.float32)
    d["b_cv2f"] = np.ascontiguousarray(bi.reshape(2, 128).T).astype(np.float32)
    return d


WEIGHT_DECLS = None  # filled by build_nc


def build_nc():
    """Emit the full Bass/Tile program for one core (2 images)."""
    nc = bacc.Bacc(target_bir_lowering=False)
    x_d = nc.dram_tensor("x", [IMGS, C1, PX], F32R, kind="ExternalInput")
    out_d = nc.dram_tensor("out", [IMGS, C1, PX], F32, kind="ExternalOutput")
    a_d = nc.dram_tensor("a_spill", [IMGS, 128, PX], BF16)

    wd = {}

    def wdecl(name, shape, dt):
        wd[name] = nc.dram_tensor(name, list(shape), dt, kind="ExternalInput")

    wdecl("w_cv1", (128, 2, 2, 128), F32R)
    wdecl("s_cv1", (128, 2), F32)
    wdecl("b_cv1", (128, 2), F32)
    for i in range(2):
        wdecl(f"w_bcv1_{i}", (128, 64), BF16)
        wdecl(f"s_bcv1_{i}", (128, 1), F32)
        wdecl(f"b_bcv1_{i}", (128, 1), F32)
        wdecl(f"w_c1_{i}", (128, 128), BF16)
        wdecl(f"s_c1_{i}", (128, 1), F32)
        wdecl(f"b_c1_{i}", (128, 1), F32)
        wdecl(f"w_kg1_{i}", (128, 32), F32)
        wdecl(f"s_kg1_{i}", (32, 1), F32)
        wdecl(f"b_kg1_{i}", (32, 1), F32)
        wdecl(f"w_kg2_{i}", (32, 72), F32)
        wdecl(f"s_kg2_{i}", (72, 1), F32)
        wdecl(f"b_kg2_{i}", (72, 1), F32)
        wdecl(f"w_c2_{i}", (128, 128), BF16)
        wdecl(f"s_c2_{i}", (128, 1), F32)
        wdecl(f"b_c2_{i}", (128, 1), F32)
        wdecl(f"w_bcv2_{i}", (128, 128), BF16)
        wdecl(f"s_bcv2_{i}", (128, 1), F32)
        wdecl(f"b_bcv2_{i}", (128, 1), F32)
    wdecl("mask72", (72, 9), F32)
    wdecl("sel", (72, 128), F32)
    wdecl("w_cv2f", (128, 4, 2, 128), BF16)
    wdecl("s_cv2f", (128, 2), F32)
    wdecl("b_cv2f", (128, 2), F32)

    with tile.TileContext(nc) as tc:
        with (
            tc.tile_pool(name="const", bufs=1) as cpool,
            tc.tile_pool(name="big", bufs=1) as big,
            tc.tile_pool(name="mid", bufs=1) as mid,
            tc.tile_pool(name="io", bufs=3) as io,
            tc.tile_pool(name="stage", bufs=2) as stg,
            tc.tile_pool(name="small", bufs=1) as sml,
            tc.tile_pool(name="psum", bufs=2, space="PSUM") as psp,
        ):
            # ---- load weights into SBUF ----
            wt = {}
            for name, dram in wd.items():
                t_ = cpool.tile(list(dram.shape), dram.dtype, tag=name)
                nc.sync.dma_start(t_[:], dram[:])
                wt[name] = t_

            # ---- persistent activations ----
            b2_t = big.tile([128, IMGS, PX], BF16, tag="b2")
            f_t = big.tile([128, IMGS, PX], BF16, tag="f")
            g_t = big.tile([128, IMGS, PX], BF16, tag="g")
            A_t = mid.tile([128, HP, WP], BF16, tag="padA")
            B_t = mid.tile([128, HP, WP], BF16, tag="padB")
            nc.gpsimd.memset(A_t[:], 0.0)
            nc.gpsimd.memset(B_t[:], 0.0)

            # =============== main cv1 (256 -> a | b2) ===============
            for img in range(IMGS):
                for (p0, gl) in GRPS:
                    xk = []
                    for kt in range(2):
                        xt = io.tile([128, 2048], F32R, tag="io")
                        nc.sync.dma_start(xt[:, :gl], x_d[img, kt * 128:(kt + 1) * 128, p0:p0 + gl])
                        xk.append(xt)
                    for mt in range(2):
                        ps = psp.tile([128, 2048], F32, tag="ps")
                        for ci in range(0, gl, 512):
                            cw = min(512, gl - ci)
                            for kt in range(2):
                                nc.tensor.matmul(
                                    ps[:, ci:ci + cw],
                                    wt["w_cv1"][:, kt, mt, :],
                                    xk[kt][:, ci:ci + cw],
                                    start=(kt == 0), stop=(kt == 1),
                                )
                        if mt == 1:  # b2 half stays in SBUF
                            nc.scalar.activation(
                                b2_t[:, img, p0:p0 + gl], ps[:, :gl], AF.Silu,
                                bias=wt["b_cv1"][:, 1:2], scale=wt["s_cv1"][:, 1:2])
                        else:        # a half -> DRAM spill (read back by final conv)
                            st = io.tile([128, 2048], BF16, tag="io")
                            nc.scalar.activation(
                                st[:, :gl], ps[:, :gl], AF.Silu,
                                bias=wt["b_cv1"][:, 0:1], scale=wt["s_cv1"][:, 0:1])
                            nc.sync.dma_start(a_d[img, :, p0:p0 + gl], st[:, :gl])

            # =============== CIB blocks ===============
            for blk in range(2):
                src = b2_t if blk == 0 else f_t
                dst = f_t if blk == 0 else g_t
                t1_t = mid.tile([128, PX], BF16, tag="t1u")
                # ---- block cv1: 128 -> 64 (2 K-passes on block 1: b2 + f) ----
                for (p0, gl) in GRPS:
                    ps = psp.tile([128, 2048], F32, tag="ps")
                    for ci in range(0, gl, 512):
                        cw = min(512, gl - ci)
                        for img in range(IMGS):
                            if blk == 0:
                                nc.tensor.matmul(
                                    ps[img * 64:(img + 1) * 64, ci:ci + cw],
                                    wt["w_bcv1_0"][:, :], b2_t[:, img, p0 + ci:p0 + ci + cw],
                                    start=True, stop=True)
                            else:
                                nc.tensor.matmul(
                                    ps[img * 64:(img + 1) * 64, ci:ci + cw],
                                    wt["w_bcv1_1"][:, :], b2_t[:, img, p0 + ci:p0 + ci + cw],
                                    start=True, stop=False)
                                nc.tensor.matmul(
                                    ps[img * 64:(img + 1) * 64, ci:ci + cw],
                                    wt["w_bcv1_1"][:, :], f_t[:, img, p0 + ci:p0 + ci + cw],
                                    start=False, stop=True)
                    nc.scalar.activation(
                        t1_t[:, p0:p0 + gl], ps[:, :gl], AF.Silu,
                        bias=wt[f"b_bcv1_{blk}"][:], scale=wt[f"s_bcv1_{blk}"][:])

                # ---- conv1: 64 -> 64 packed, epilogue writes padded A + pooled sums ----
                pooled_parts = sml.tile([128, 4], F32, tag="pparts")
                for rg, (r0, nr, crows) in enumerate(RGRPS):
                    ps = psp.tile([128, 2048], F32, tag="ps")
                    nch = nr // crows
                    cw = crows * 80
                    for ci in range(nch):
                        pp = r0 * 80 + ci * cw
                        nc.tensor.matmul(
                            ps[:, ci * 512:ci * 512 + cw],
                            wt[f"w_c1_{blk}"][:, :], t1_t[:, pp:pp + cw],
                            start=True, stop=True)
                    ps_v = ps[:, :nch * 512].rearrange("p (c x) -> p c x", c=nch)[:, :, :cw]
                    ps_v = ps_v.rearrange("p c (r w) -> p c r w", w=80)
                    nc.scalar.activation(
                        A_t[:, 1 + r0:1 + r0 + nr, 2:82].rearrange("p (c r) w -> p c r w", r=crows),
                        ps_v, AF.Silu,
                        bias=wt[f"b_c1_{blk}"][:], scale=wt[f"s_c1_{blk}"][:],
                        accum_out=pooled_parts[:, rg:rg + 1])
                # shifted copy (interior at cols 1..80) for bf16 alignment of dx=+-1 taps
                nc.vector.tensor_copy(B_t[:, 1:81, 1:81], A_t[:, 1:81, 2:82])

                # ---- kernel generation ----
                pooled = sml.tile([128, 1], F32, tag="pooled")
                nc.vector.tensor_reduce(pooled[:], pooled_parts[:], op=mybir.AluOpType.add,
                                        axis=mybir.AxisListType.X)
                ps = psp.tile([128, 2048], F32, tag="ps")
                nc.tensor.matmul(ps[0:32, 0:1], wt[f"w_kg1_{blk}"][:, :],
                                 pooled[:], start=True, stop=True)
                z0 = sml.tile([32, 1], F32, tag="z0")
                nc.scalar.activation(z0[:], ps[0:32, 0:1], AF.Silu,
                                     bias=wt[f"b_kg1_{blk}"][:], scale=wt[f"s_kg1_{blk}"][:])
                z1 = sml.tile([32, 1], F32, tag="z1")
                nc.scalar.activation(z1[:], z0[:], AF.Silu)
                ps2 = psp.tile([128, 2048], F32, tag="ps")
                nc.tensor.matmul(ps2[0:72, 0:1], wt[f"w_kg2_{blk}"][:, :],
                                 z1[:], start=True, stop=True)
                kh = sml.tile([72, 1], F32, tag="kh")
                nc.scalar.activation(kh[:], ps2[0:72, 0:1], AF.Silu,
                                     bias=wt[f"b_kg2_{blk}"][:], scale=wt[f"s_kg2_{blk}"][:])
                D_t = sml.tile([72, 9], F32, tag="D")
                nc.vector.tensor_scalar_mul(D_t[:], wt["mask72"][:], kh[:])
                ps3 = psp.tile([128, 2048], F32, tag="ps")
                nc.tensor.matmul(ps3[:, 0:9], wt["sel"][:, :], D_t[:],
                                 start=True, stop=True)
                K9 = sml.tile([128, 9], F32, tag="K9")
                nc.scalar.activation(K9[:], ps3[:, 0:9], AF.Copy)

                # ---- depthwise 3x3: 9 taps on DVE, two 40-row halves ----
                S1_t = mid.tile([128, PX], BF16, tag="s1")
                tmp_t = mid.tile([128, 3200], BF16, tag="tmp")
                S1v = S1_t[:].rearrange("p (h w) -> p h w", w=80)
                tmpv = tmp_t[:].rearrange("p (h w) -> p h w", w=80)
                for r0 in (0, 40):
                    for ti, (dy, dx) in enumerate(TAPS):
                        buf, c0 = (A_t, 2) if dx == 0 else (B_t, 1)
                        view = buf[:, 1 + r0 + dy:41 + r0 + dy, c0 + dx:c0 + dx + 80]
                        tgt = S1v[:, r0:r0 + 40, :] if ti == 0 else tmpv
                        nc.vector.tensor_scalar_mul(tgt, view, K9[:, ti:ti + 1])
                        if ti > 0:
                            nc.vector.tensor_add(S1_t[:, r0 * 80:(r0 + 40) * 80],
                                                 S1_t[:, r0 * 80:(r0 + 40) * 80], tmp_t[:])

                # ---- conv2: 64 -> 64 packed ----
                u_t = mid.tile([128, PX], BF16, tag="t1u")
                for (p0, gl) in GRPS:
                    ps = psp.tile([128, 2048], F32, tag="ps")
                    for ci in range(0, gl, 512):
                        cw = min(512, gl - ci)
                        nc.tensor.matmul(ps[:, ci:ci + cw], wt[f"w_c2_{blk}"][:, :],
                                         S1_t[:, p0 + ci:p0 + ci + cw], start=True, stop=True)
                    nc.scalar.activation(
                        u_t[:, p0:p0 + gl], ps[:, :gl], AF.Silu,
                        bias=wt[f"b_c2_{blk}"][:], scale=wt[f"s_c2_{blk}"][:])

                # ---- block cv2: 64 -> 128 per image ----
                for img in range(IMGS):
                    for (p0, gl) in GRPS:
                        ps = psp.tile([128, 2048], F32, tag="ps")
                        for ci in range(0, gl, 512):
                            cw = min(512, gl - ci)
                            nc.tensor.matmul(
                                ps[:, ci:ci + cw],
                                wt[f"w_bcv2_{blk}"][img * 64:(img + 1) * 64, :],
                                u_t[img * 64:(img + 1) * 64, p0 + ci:p0 + ci + cw],
                                start=True, stop=True)
                        nc.scalar.activation(
                            dst[:, img, p0:p0 + gl], ps[:, :gl], AF.Silu,
                            bias=wt[f"b_bcv2_{blk}"][:], scale=wt[f"s_bcv2_{blk}"][:])

            # =============== final cv2 (512 -> 256, residual-folded) ===============
            for img in range(IMGS):
                for (p0, gl) in GRPS:
                    at = io.tile([128, 2048], BF16, tag="io")
                    nc.sync.dma_start(at[:, :gl], a_d[img, :, p0:p0 + gl])
                    ksrc = [at[:, 0:gl],
                            b2_t[:, img, p0:p0 + gl],
                            f_t[:, img, p0:p0 + gl],
                            g_t[:, img, p0:p0 + gl]]
                    for mt in range(2):
                        ps = psp.tile([128, 2048], F32, tag="ps")
                        for ci in range(0, gl, 512):
                            cw = min(512, gl - ci)
                            for kt in range(4):
                                nc.tensor.matmul(
                                    ps[:, ci:ci + cw], wt["w_cv2f"][:, kt, mt, :],
                                    ksrc[kt][:, ci:ci + cw],
                                    start=(kt == 0), stop=(kt == 3))
                        st = stg.tile([128, 2048], F32, tag="stf")
                        nc.scalar.activation(
                            st[:, :gl], ps[:, :gl], AF.Silu,
                            bias=wt["b_cv2f"][:, mt:mt + 1], scale=wt["s_cv2f"][:, mt:mt + 1])
                        nc.sync.dma_start(out_d[img, mt * 128:(mt + 1) * 128, p0:p0 + gl],
                                          st[:, :gl])

    nc.finalize()
    return nc


def kernel(x, params):
    x = np.asarray(x, np.float32)
    w = prep_weights(params)
    nc = build_nc()
    in_maps = []
    for c in range(NCORES):
        m = {"x": np.ascontiguousarray(x[c * IMGS:(c + 1) * IMGS].reshape(IMGS, C1, PX))}
        m.update(w)
        in_maps.append(m)
    res = run_bass_kernel_spmd(nc, in_maps, list(range(NCORES)))
    out = np.stack([res.results[c]["out"] for c in range(NCORES)])
    return out.reshape(B, C1, H, W).astype(np.float32)
